# revision 23
# baseline (speedup 1.0000x reference)
"""Causal self-attention on 8 trn2 NeuronCores.

Sharding: DP4 (batch) x TP2 (head groups of 8). Core c -> batch c//2,
head group c%2. Each core computes qkv^T for its 512 channels, causal
attention for its 8 heads over all T=2048 queries, and a partial
projection y_partial = O_g @ W_proj[rows_g] (+ b_proj on group 0).
Host sums the two partials per batch and transposes (kernel emits y^T).

v2: bf16 datapath (x, W, Q/K/V, P, O in bf16; PSUM accum f32),
attention restructured j-granular with a software pipeline that keeps
PE busy: QKV work units for head-pairs 1-3 and V tiles 4-15 are
interleaved into the attention i-loop, filling the PE gaps left while
the ACT engine runs Exp. The v-bias is folded into V via a K=1
ones-row matmul (softmax rows sum to 1 after normalization, so
biasing V pre-attention equals biasing O post-normalize). Softmax
row-sums come from a ones-column appended to V; no max-subtraction
(scores ~ N(0,1), exp can't overflow).
"""
import sys

sys.path.insert(0, "/opt/trn_rl_repo")

import numpy as np

import concourse.bass as bass
import concourse.tile as tile
from concourse import bacc, mybir

f32 = mybir.dt.float32
bf16 = mybir.dt.bfloat16
AFT = mybir.ActivationFunctionType

N_CORES = 8
B, T, C = 4, 2048, 1024
H, HD = 16, 64            # total heads, head dim
HPC = 8                   # heads per core
CPC = 512                 # channels per core (q, k or v)
NS = T // 512             # 4 t-slices of 512
NC_T = C // 128           # 8 C-tiles (contraction)
SCALE = 1.0 / np.sqrt(HD)
INTERLEAVE = True


def build_nc(repeat: int = 1):
    nc = bacc.Bacc("TRN2", target_bir_lowering=False, debug=False,
                   num_devices=N_CORES)

    xb_d = nc.dram_tensor("xb", [C, T], bf16, kind="ExternalInput")
    wq_d = nc.dram_tensor("wq", [C, CPC], bf16, kind="ExternalInput")
    wk_d = nc.dram_tensor("wk", [C, CPC], bf16, kind="ExternalInput")
    wv_d = nc.dram_tensor("wv", [C, CPC], bf16, kind="ExternalInput")
    bqk_d = nc.dram_tensor("bqk", [128, 8], f32, kind="ExternalInput")
    bv_d = nc.dram_tensor("bv", [1, CPC], bf16, kind="ExternalInput")
    wp_d = nc.dram_tensor("wp", [CPC, C], bf16, kind="ExternalInput")
    bp_d = nc.dram_tensor("bp", [128, 8], f32, kind="ExternalInput")
    masks_d = nc.dram_tensor("masks", [128, 4 * 1024], bf16,
                             kind="ExternalInput")
    yt_d = nc.dram_tensor("yT", [C, T], f32, kind="ExternalOutput")

    with tile.TileContext(nc) as tc:
        pers_cm = tc.tile_pool(name="pers", bufs=1)
        pers = pers_cm.__enter__()
        tiles = _load_consts(nc, pers, wq_d, wk_d, wv_d, bqk_d, bv_d,
                             wp_d, bp_d, masks_d)

        if repeat == 1:
            _build_body(nc, tc, tiles, xb_d, yt_d, 0, 0, skew=False)
        else:
            # Software-pipeline phase C across iterations: the loop body
            # is double-emitted with ping-ponged ot buffers; each half
            # projects the PREVIOUS half's attention output from inside
            # the attention i-loop, keeping PE saturated (and ramped).
            assert repeat % 2 == 0
            with tc.For_i(0, repeat // 2, 1):
                _build_body(nc, tc, tiles, xb_d, yt_d, 0, 1, skew=True)
                _build_body(nc, tc, tiles, xb_d, yt_d, 1, 0, skew=True)
        pers_cm.__exit__(None, None, None)
    nc.compile()
    return nc


def _load_consts(nc, pers, wq_d, wk_d, wv_d, bqk_d, bv_d, wp_d, bp_d,
                 masks_d):
    """Allocate persistent tiles; DMA the iteration-invariant ones
    (weights, biases, masks) once, outside the repeat loop."""
    t = {}
    t["x"] = pers.tile([128, NC_T * T], bf16, name="x")
    t["wq"] = pers.tile([128, NC_T * CPC], bf16, name="wq")
    t["wk"] = pers.tile([128, NC_T * CPC], bf16, name="wk")
    t["wv"] = pers.tile([128, NC_T * CPC], bf16, name="wv")
    t["wp"] = pers.tile([128, 4 * C], bf16, name="wp")
    t["bqk"] = pers.tile([128, 8], f32, name="bqk")
    t["bp"] = pers.tile([128, 8], f32, name="bp")
    t["bv"] = pers.tile([1, CPC], bf16, name="bv")
    t["ones1"] = pers.tile([1, 128], bf16, name="ones1")
    t["masks"] = pers.tile([128, 4 * 1024], bf16, name="masks")
    t["qt"] = [pers.tile([128, T], bf16, name=f"qt{i}") for i in range(4)]
    t["kt"] = [pers.tile([128, T], bf16, name=f"kt{i}") for i in range(4)]
    t["vaug"] = [pers.tile([128, HPC * 65], bf16, name=f"vaug{i}")
                 for i in range(16)]
    t["ot"] = [[pers.tile([128, T], bf16, name=f"ot{b}_{i}")
                for i in range(4)] for b in range(2)]

    nc.gpsimd.memset(t["ones1"][:], 1.0)
    for i in range(16):
        onescol = t["vaug"][i][:].rearrange("p (h w) -> p h w", w=65)[:, :, 64:65]
        nc.gpsimd.memset(onescol, 1.0)

    nc.sync.dma_start(
        t["wq"][:].rearrange("p (c w) -> p c w", c=NC_T),
        wq_d.ap().rearrange("(c p) w -> p c w", p=128))
    nc.sync.dma_start(
        t["wk"][:].rearrange("p (c w) -> p c w", c=NC_T),
        wk_d.ap().rearrange("(c p) w -> p c w", p=128))
    nc.gpsimd.dma_start(
        t["wv"][:].rearrange("p (c w) -> p c w", c=NC_T),
        wv_d.ap().rearrange("(c p) w -> p c w", p=128))
    nc.gpsimd.dma_start(t["masks"][:], masks_d.ap())
    nc.scalar.dma_start(
        t["wp"][:].rearrange("p (c w) -> p c w", c=4),
        wp_d.ap().rearrange("(c p) w -> p c w", p=128))
    nc.scalar.dma_start(t["bqk"][:], bqk_d.ap())
    nc.scalar.dma_start(t["bv"][:], bv_d.ap())
    nc.scalar.dma_start(t["bp"][:], bp_d.ap())
    return t


def _build_body(nc, tc, tiles, xb_d, yt_d, cur, prev, skew):
    x_sb = tiles["x"]
    wq_sb, wk_sb, wv_sb, wp_sb = (tiles["wq"], tiles["wk"], tiles["wv"],
                                  tiles["wp"])
    bqk, bp, bv, ones1, masks = (tiles["bqk"], tiles["bp"], tiles["bv"],
                                 tiles["ones1"], tiles["masks"])
    qt, kt, vaug = tiles["qt"], tiles["kt"], tiles["vaug"]
    ot = tiles["ot"][cur]
    ot_prev = tiles["ot"][prev]

    # ---------- per-iteration input DMA: x, spread across queues ----------
    def xslice(s):
        return (x_sb[:].rearrange("p (c t) -> p c t", c=NC_T)[:, :, 512 * s:512 * s + 512],
                xb_d.ap().rearrange("(c p) t -> p c t", p=128)[:, :, 512 * s:512 * s + 512])

    nc.sync.dma_start(*xslice(0))
    nc.gpsimd.dma_start(*xslice(1))
    nc.scalar.dma_start(*xslice(2))
    nc.sync.dma_start(*xslice(3))

    with tc.tile_pool(name="pst", bufs=2, space="PSUM") as pst_pool, \
         tc.tile_pool(name="pot", bufs=4, space="PSUM") as pot_pool, \
         tc.tile_pool(name="pt", bufs=4) as pt_pool, \
         tc.tile_pool(name="rl", bufs=4) as rl_pool, \
         tc.tile_pool(name="rlb", bufs=4) as rlb_pool:

        # ----- phase-A work units (one PSUM group each) -----
        def qk_unit(qk, g, s):
            def emit():
                w_sb = wq_sb if qk == 0 else wk_sb
                dst = (qt if qk == 0 else kt)[g]
                ps = pst_pool.tile([128, 1024], f32, name="pst")[:, 0:512]
                for ci in range(NC_T):
                    nc.tensor.matmul(
                        ps[:], w_sb[:, 512 * ci + 128 * g:512 * ci + 128 * g + 128],
                        x_sb[:, T * ci + 512 * s:T * ci + 512 * s + 512],
                        start=(ci == 0), stop=(ci == NC_T - 1))
                nc.vector.tensor_scalar_add(
                    dst[:, 512 * s:512 * s + 512], ps[:],
                    bqk[:, 4 * qk + g:4 * qk + g + 1])
            return emit

        def v_unit(t):
            def emit():
                s, tt = divmod(t, 4)
                ps = pst_pool.tile([128, 1024], f32, name="pst")[:, 0:512]
                for ci in range(NC_T):
                    nc.tensor.matmul(
                        ps[:],
                        x_sb[:, T * ci + 128 * t:T * ci + 128 * t + 128],
                        wv_sb[:, 512 * ci:512 * ci + 512],
                        start=(ci == 0), stop=False)
                nc.tensor.matmul(ps[:], ones1[:, 0:128], bv[:],
                                 start=False, stop=True)
                dst = vaug[t][:].rearrange("p (h w) -> p h w", w=65)[:, :, 0:64]
                nc.vector.tensor_copy(
                    dst, ps[:].rearrange("p (h w) -> p h w", w=64))
            return emit

        # Slice s=0 of Q/K head-pair 0 and the first 4 V tiles must
        # precede B; everything else is paced into B's i-loop so PE
        # stays busy while ACT runs Exp.
        qk_unit(0, 0, 0)()
        qk_unit(1, 0, 0)()
        for t in range(4):
            v_unit(t)()
        pending = []
        for s in range(1, NS):      # rest of head-pair 0 Q/K (6 units)
            pending.append(qk_unit(0, 0, s))
            pending.append(qk_unit(1, 0, s))
        pending += [v_unit(t) for t in range(4, 16)]
        for g in range(1, 4):
            for qk in range(2):
                for s in range(NS):
                    pending.append(qk_unit(qk, g, s))
        pending.reverse()           # pop() from the front of the list
        if not INTERLEAVE:
            while pending:
                pending.pop()()

        # ----- phase-C work units (skew mode: project ot_prev) -----
        yt_pool_cm = tc.tile_pool(name="yt", bufs=4)
        yt_pool = yt_pool_cm.__enter__()

        def c_unit(g, s):
            def emit():
                ps = pst_pool.tile([128, 1024], f32, name="pst")[:, 0:512]
                for ci in range(4):
                    nc.tensor.matmul(
                        ps[:], wp_sb[:, C * ci + 128 * g:C * ci + 128 * g + 128],
                        ot_prev[ci][:, 512 * s:512 * s + 512],
                        start=(ci == 0), stop=(ci == 3))
                yt = yt_pool.tile([128, 512], f32, name="yt")
                nc.vector.tensor_scalar_add(yt[:], ps[:], bp[:, g:g + 1])
                q_eng = (nc.sync, nc.gpsimd)[(4 * g + s) % 2]
                q_eng.dma_start(
                    yt_d.ap()[128 * g:128 * g + 128, 512 * s:512 * s + 512],
                    yt[:])
            return emit

        c_pending = []
        if skew:
            for g in range(8):
                for s in range(NS):
                    c_pending.append(c_unit(g, s))
            c_pending.reverse()

        # pop pacing: steps 1-18 one unit/step (QK0 rest + V tiles,
        # needed early), steps 19-35 every 2nd (QK1 before B(1)@40),
        # then every 3rd until exhausted. C units fill every 5th step.
        def want_popped(step):
            if step <= 18:
                return step
            if step <= 35:
                return 18 + (step - 17) // 2
            return 27 + (step - 33) // 3

        # ----- phase B: attention, j-granular, software-pipelined -----
        step = 0
        popped = 0
        for hp in range(4):
            for j in range(4):
                po = [pot_pool.tile([65, 512], f32, name="pot")
                      for _ in range(2)]
                for i in range(4 * j + 4):
                    step += 1
                    pst = pst_pool.tile([128, 1024], f32, name="pst")
                    for hl in range(2):
                        rows = slice(64 * hl, 64 * hl + 64)
                        nc.tensor.matmul(
                            pst[:, 512 * hl:512 * hl + 512],
                            kt[hp][rows, 128 * i:128 * i + 128],
                            qt[hp][rows, 512 * j:512 * j + 512],
                            start=True, stop=True)
                    ptile = pt_pool.tile([128, 1024], bf16, name="pt")
                    nc.scalar.activation(ptile[:], pst[:], AFT.Exp)
                    if i // 4 == j:
                        o = i % 4
                        nc.vector.tensor_mul(
                            ptile[:], ptile[:],
                            masks[:, 1024 * o:1024 * o + 1024])
                    # interleaved phase-A/C units: land on PE between
                    # the S and AV matmuls, covering the Exp latency.
                    if pending and popped < want_popped(step):
                        pending.pop()()
                        popped += 1
                    if c_pending and step % 5 == 3:
                        c_pending.pop()()
                    for hl in range(2):
                        h = 2 * hp + hl
                        nc.tensor.matmul(
                            po[hl][:], vaug[i][:, 65 * h:65 * h + 65],
                            ptile[:, 512 * hl:512 * hl + 512],
                            start=(i == 0), stop=(i == 4 * j + 3))
                for hl in range(2):
                    rl = rl_pool.tile([1, 512], f32, name="rl")
                    nc.vector.reciprocal(rl[:], po[hl][64:65, :])
                    rlb = rlb_pool.tile([64, 512], f32, name="rlb")
                    nc.gpsimd.partition_broadcast(rlb[:], rl[:])
                    nc.vector.tensor_mul(
                        ot[hp][64 * hl:64 * hl + 64, 512 * j:512 * j + 512],
                        po[hl][0:64, :], rlb[:])
        while pending:
            pending.pop()()
        while c_pending:
            c_pending.pop()()

        if not skew:
            # ---------- phase C inline (repeat==1 path) ----------
            for g in range(8):
                for s in range(NS):
                    ps = pst_pool.tile([128, 1024], f32, name="pst")[:, 0:512]
                    for ci in range(4):
                        nc.tensor.matmul(
                            ps[:], wp_sb[:, C * ci + 128 * g:C * ci + 128 * g + 128],
                            ot[ci][:, 512 * s:512 * s + 512],
                            start=(ci == 0), stop=(ci == 3))
                    yt = yt_pool.tile([128, 512], f32, name="yt")
                    nc.scalar.activation(yt[:], ps[:], AFT.Identity,
                                         bias=bp[:, g:g + 1])
                    q_eng = (nc.sync, nc.gpsimd)[(4 * g + s) % 2]
                    q_eng.dma_start(
                        yt_d.ap()[128 * g:128 * g + 128, 512 * s:512 * s + 512],
                        yt[:])
        yt_pool_cm.__exit__(None, None, None)


def make_inputs(x, W_attn, b_attn, W_proj, b_proj):
    """Host-side sharding: per-core input dicts (bf16 datapath)."""
    import ml_dtypes
    bf = ml_dtypes.bfloat16
    x = np.asarray(x, np.float32)
    W_attn = np.asarray(W_attn, np.float32)
    b_attn = np.asarray(b_attn, np.float32)
    W_proj = np.asarray(W_proj, np.float32)
    b_proj = np.asarray(b_proj, np.float32)

    # masks[kk, 1024*o + 512*hl + qq] = 1 if kk + 128*o <= qq (dup per hl)
    kk = np.arange(128)[:, None]
    qq = np.arange(512)[None, :]
    masks = np.zeros((128, 4 * 1024), np.float32)
    for o in range(4):
        m = (kk + 128 * o <= qq).astype(np.float32)
        masks[:, 1024 * o:1024 * o + 512] = m
        masks[:, 1024 * o + 512:1024 * (o + 1)] = m

    in_maps = []
    for core in range(N_CORES):
        b, g = divmod(core, 2)
        qcols = np.arange(CPC * g, CPC * g + CPC)
        wq = W_attn[:, qcols] * SCALE
        wk = W_attn[:, C + qcols]
        wv = W_attn[:, 2 * C + qcols]
        bq = b_attn[qcols] * SCALE
        bk = b_attn[C + qcols]
        bvv = b_attn[2 * C + qcols]
        bqk = np.concatenate([bq.reshape(4, 128).T, bk.reshape(4, 128).T],
                             axis=1)                     # [128, 8]
        wp = W_proj[CPC * g:CPC * g + CPC, :]
        bpv = (b_proj if g == 0 else np.zeros(C, np.float32))
        in_maps.append({
            "xb": np.ascontiguousarray(x[b].T).astype(bf),
            "wq": np.ascontiguousarray(wq).astype(bf),
            "wk": np.ascontiguousarray(wk).astype(bf),
            "wv": np.ascontiguousarray(wv).astype(bf),
            "bqk": np.ascontiguousarray(bqk),
            "bv": np.ascontiguousarray(bvv.reshape(1, CPC)).astype(bf),
            "wp": np.ascontiguousarray(wp).astype(bf),
            "bp": np.ascontiguousarray(bpv.reshape(8, 128).T),
            "masks": masks.astype(bf),
        })
    return in_maps


def unshard(results):
    """Combine per-core yT partials into [B, T, C] output."""
    out = np.empty((B, T, C), np.float32)
    for b in range(B):
        yt = results[2 * b]["yT"] + results[2 * b + 1]["yT"]
        out[b] = yt.T
    return out


_nc_cache = {}


def kernel(x, W_attn, b_attn, W_proj, b_proj):
    from concourse.bass_utils import run_bass_kernel_spmd
    if "nc" not in _nc_cache:
        _nc_cache["nc"] = build_nc(repeat=1)
    nc = _nc_cache["nc"]
    in_maps = make_inputs(x, W_attn, b_attn, W_proj, b_proj)
    res = run_bass_kernel_spmd(nc, in_maps, core_ids=list(range(N_CORES)),
                               trace=False)
    return unshard(res.results)


# revision 28
# speedup vs baseline: 1.1738x; 1.1738x over previous
"""Causal self-attention on 8 trn2 NeuronCores.

Sharding: DP4 (batch) x TP2 (head groups of 8). Core c -> batch c//2,
head group c%2. Each core computes qkv^T for its 512 channels, causal
attention for its 8 heads over all T=2048 queries, and a partial
projection y_partial = O_g @ W_proj[rows_g] (+ b_proj on group 0).
Host sums the two partials per batch and transposes (kernel emits y^T).

v2: bf16 datapath (x, W, Q/K/V, P, O in bf16; PSUM accum f32),
attention restructured j-granular with a software pipeline that keeps
PE busy: QKV work units for head-pairs 1-3 and V tiles 4-15 are
interleaved into the attention i-loop, filling the PE gaps left while
the ACT engine runs Exp. The v-bias is folded into V via a K=1
ones-row matmul (softmax rows sum to 1 after normalization, so
biasing V pre-attention equals biasing O post-normalize). Softmax
row-sums come from a ones-column appended to V; no max-subtraction
(scores ~ N(0,1), exp can't overflow).
"""
import sys

sys.path.insert(0, "/opt/trn_rl_repo")

import numpy as np

import concourse.bass as bass
import concourse.tile as tile
from concourse import bacc, mybir

f32 = mybir.dt.float32
bf16 = mybir.dt.bfloat16
AFT = mybir.ActivationFunctionType

N_CORES = 8
B, T, C = 4, 2048, 1024
H, HD = 16, 64            # total heads, head dim
HPC = 8                   # heads per core
CPC = 512                 # channels per core (q, k or v)
NS = T // 512             # 4 t-slices of 512
NC_T = C // 128           # 8 C-tiles (contraction)
SCALE = 1.0 / np.sqrt(HD)
INTERLEAVE = True


def build_nc(repeat: int = 1):
    nc = bacc.Bacc("TRN2", target_bir_lowering=False, debug=False,
                   num_devices=N_CORES)

    xb_d = nc.dram_tensor("xb", [C, T], bf16, kind="ExternalInput")
    wq_d = nc.dram_tensor("wq", [C, CPC], bf16, kind="ExternalInput")
    wk_d = nc.dram_tensor("wk", [C, CPC], bf16, kind="ExternalInput")
    wv_d = nc.dram_tensor("wv", [C, CPC], bf16, kind="ExternalInput")
    bqk_d = nc.dram_tensor("bqk", [128, 8], f32, kind="ExternalInput")
    bv_d = nc.dram_tensor("bv", [1, CPC], bf16, kind="ExternalInput")
    wp_d = nc.dram_tensor("wp", [CPC, C], bf16, kind="ExternalInput")
    bp_d = nc.dram_tensor("bp", [128, 8], f32, kind="ExternalInput")
    masks_d = nc.dram_tensor("masks", [128, 4 * 1024], bf16,
                             kind="ExternalInput")
    yt_d = nc.dram_tensor("yT", [C, T], f32, kind="ExternalOutput")

    with tile.TileContext(nc) as tc:
        pers_cm = tc.tile_pool(name="pers", bufs=1)
        pers = pers_cm.__enter__()
        tiles = _load_consts(nc, pers, wq_d, wk_d, wv_d, bqk_d, bv_d,
                             wp_d, bp_d, masks_d)

        if repeat == 1:
            _build_body(nc, tc, tiles, xb_d, yt_d, 0, 0, skew=False)
        else:
            # Software-pipeline phase C across iterations: the loop body
            # is double-emitted with ping-ponged ot buffers; each half
            # projects the PREVIOUS half's attention output from inside
            # the attention i-loop, keeping PE saturated (and ramped).
            assert repeat % 2 == 0
            with tc.For_i(0, repeat // 2, 1):
                _build_body(nc, tc, tiles, xb_d, yt_d, 0, 1, skew=True)
                _build_body(nc, tc, tiles, xb_d, yt_d, 1, 0, skew=True)
        pers_cm.__exit__(None, None, None)
    nc.compile()
    return nc


def _load_consts(nc, pers, wq_d, wk_d, wv_d, bqk_d, bv_d, wp_d, bp_d,
                 masks_d):
    """Allocate persistent tiles; DMA the iteration-invariant ones
    (weights, biases, masks) once, outside the repeat loop."""
    t = {}
    t["x"] = pers.tile([128, NC_T * T], bf16, name="x")
    t["wq"] = pers.tile([128, NC_T * CPC], bf16, name="wq")
    t["wk"] = pers.tile([128, NC_T * CPC], bf16, name="wk")
    t["wv"] = pers.tile([128, NC_T * CPC], bf16, name="wv")
    t["wp"] = pers.tile([128, 4 * C], bf16, name="wp")
    t["bqk"] = pers.tile([128, 8], f32, name="bqk")
    t["bp"] = pers.tile([128, 8], f32, name="bp")
    t["bv"] = pers.tile([1, CPC], bf16, name="bv")
    t["ones1"] = pers.tile([1, 128], bf16, name="ones1")
    t["masks"] = pers.tile([128, 4 * 1024], bf16, name="masks")
    t["qt"] = [pers.tile([128, T], bf16, name=f"qt{i}") for i in range(4)]
    t["kt"] = [pers.tile([128, T], bf16, name=f"kt{i}") for i in range(4)]
    t["vaug"] = [pers.tile([128, HPC * 65], bf16, name=f"vaug{i}")
                 for i in range(16)]
    t["ot"] = [[pers.tile([128, T], bf16, name=f"ot{b}_{i}")
                for i in range(4)] for b in range(2)]

    nc.gpsimd.memset(t["ones1"][:], 1.0)
    for i in range(16):
        onescol = t["vaug"][i][:].rearrange("p (h w) -> p h w", w=65)[:, :, 64:65]
        nc.gpsimd.memset(onescol, 1.0)

    nc.sync.dma_start(
        t["wq"][:].rearrange("p (c w) -> p c w", c=NC_T),
        wq_d.ap().rearrange("(c p) w -> p c w", p=128))
    nc.sync.dma_start(
        t["wk"][:].rearrange("p (c w) -> p c w", c=NC_T),
        wk_d.ap().rearrange("(c p) w -> p c w", p=128))
    nc.gpsimd.dma_start(
        t["wv"][:].rearrange("p (c w) -> p c w", c=NC_T),
        wv_d.ap().rearrange("(c p) w -> p c w", p=128))
    nc.gpsimd.dma_start(t["masks"][:], masks_d.ap())
    nc.scalar.dma_start(
        t["wp"][:].rearrange("p (c w) -> p c w", c=4),
        wp_d.ap().rearrange("(c p) w -> p c w", p=128))
    nc.scalar.dma_start(t["bqk"][:], bqk_d.ap())
    nc.scalar.dma_start(t["bv"][:], bv_d.ap())
    nc.scalar.dma_start(t["bp"][:], bp_d.ap())
    return t


def _build_body(nc, tc, tiles, xb_d, yt_d, cur, prev, skew):
    x_sb = tiles["x"]
    wq_sb, wk_sb, wv_sb, wp_sb = (tiles["wq"], tiles["wk"], tiles["wv"],
                                  tiles["wp"])
    bqk, bp, bv, ones1, masks = (tiles["bqk"], tiles["bp"], tiles["bv"],
                                 tiles["ones1"], tiles["masks"])
    qt, kt, vaug = tiles["qt"], tiles["kt"], tiles["vaug"]
    ot = tiles["ot"][cur]
    ot_prev = tiles["ot"][prev]

    # ---------- per-iteration input DMA: x, spread across queues ----------
    def xslice(s):
        return (x_sb[:].rearrange("p (c t) -> p c t", c=NC_T)[:, :, 512 * s:512 * s + 512],
                xb_d.ap().rearrange("(c p) t -> p c t", p=128)[:, :, 512 * s:512 * s + 512])

    nc.sync.dma_start(*xslice(0))
    nc.sync.dma_start(*xslice(1))
    nc.sync.dma_start(*xslice(2))
    nc.sync.dma_start(*xslice(3))

    with tc.tile_pool(name="pst", bufs=2, space="PSUM") as pst_pool, \
         tc.tile_pool(name="pot", bufs=4, space="PSUM") as pot_pool, \
         tc.tile_pool(name="pt", bufs=4) as pt_pool, \
         tc.tile_pool(name="rl", bufs=4) as rl_pool, \
         tc.tile_pool(name="rlb", bufs=4) as rlb_pool:

        # ----- phase-A work units (one PSUM group each) -----
        def qk_unit(qk, g, s):
            def emit():
                w_sb = wq_sb if qk == 0 else wk_sb
                dst = (qt if qk == 0 else kt)[g]
                ps = pst_pool.tile([128, 1024], f32, name="pst")[:, 0:512]
                for ci in range(NC_T):
                    nc.tensor.matmul(
                        ps[:], w_sb[:, 512 * ci + 128 * g:512 * ci + 128 * g + 128],
                        x_sb[:, T * ci + 512 * s:T * ci + 512 * s + 512],
                        start=(ci == 0), stop=(ci == NC_T - 1))
                nc.vector.tensor_scalar_add(
                    dst[:, 512 * s:512 * s + 512], ps[:],
                    bqk[:, 4 * qk + g:4 * qk + g + 1])
            return emit

        def v_unit(t):
            def emit():
                s, tt = divmod(t, 4)
                ps = pst_pool.tile([128, 1024], f32, name="pst")[:, 0:512]
                for ci in range(NC_T):
                    nc.tensor.matmul(
                        ps[:],
                        x_sb[:, T * ci + 128 * t:T * ci + 128 * t + 128],
                        wv_sb[:, 512 * ci:512 * ci + 512],
                        start=(ci == 0), stop=False)
                nc.tensor.matmul(ps[:], ones1[:, 0:128], bv[:],
                                 start=False, stop=True)
                dst = vaug[t][:].rearrange("p (h w) -> p h w", w=65)[:, :, 0:64]
                nc.vector.tensor_copy(
                    dst, ps[:].rearrange("p (h w) -> p h w", w=64))
            return emit

        # Slice s=0 of Q/K head-pair 0 and the first 4 V tiles must
        # precede B; everything else is paced into B's i-loop so PE
        # stays busy while ACT runs Exp.
        qk_unit(0, 0, 0)()
        qk_unit(1, 0, 0)()
        for t in range(4):
            v_unit(t)()
        pending = []
        for s in range(1, NS):      # rest of head-pair 0 Q/K (6 units)
            pending.append(qk_unit(0, 0, s))
            pending.append(qk_unit(1, 0, s))
        pending += [v_unit(t) for t in range(4, 16)]
        for g in range(1, 4):
            for qk in range(2):
                for s in range(NS):
                    pending.append(qk_unit(qk, g, s))
        pending.reverse()           # pop() from the front of the list
        if not INTERLEAVE:
            while pending:
                pending.pop()()

        # ----- phase-C work units (skew mode: project ot_prev) -----
        yt_pool_cm = tc.tile_pool(name="yt", bufs=4)
        yt_pool = yt_pool_cm.__enter__()

        def c_unit(g, s):
            def emit():
                ps = pst_pool.tile([128, 1024], f32, name="pst")[:, 0:512]
                for ci in range(4):
                    nc.tensor.matmul(
                        ps[:], wp_sb[:, C * ci + 128 * g:C * ci + 128 * g + 128],
                        ot_prev[ci][:, 512 * s:512 * s + 512],
                        start=(ci == 0), stop=(ci == 3))
                yt = yt_pool.tile([128, 512], f32, name="yt")
                nc.scalar.activation(yt[:], ps[:], AFT.Identity,
                                     bias=bp[:, g:g + 1])
                nc.gpsimd.dma_start(
                    yt_d.ap()[128 * g:128 * g + 128, 512 * s:512 * s + 512],
                    yt[:])
            return emit

        c_pending = []
        if skew:
            for g in range(8):
                for s in range(NS):
                    c_pending.append(c_unit(g, s))
            c_pending.reverse()

        # pop pacing: steps 1-18 one unit/step (QK0 rest + V tiles,
        # needed early), steps 19-35 every 2nd (QK1 before B(1)@40),
        # then every 3rd until exhausted. C units fill every 5th step.
        def want_popped(step):
            if step <= 18:
                return step
            if step <= 35:
                return 18 + (step - 17) // 2
            return 27 + (step - 33) // 3

        # ----- phase B: attention, j-granular, software-pipelined -----
        step = 0
        popped = 0
        for hp in range(4):
            for j in range(4):
                po = [pot_pool.tile([65, 512], f32, name="pot")
                      for _ in range(2)]
                for i in range(4 * j + 4):
                    step += 1
                    pst = pst_pool.tile([128, 1024], f32, name="pst")
                    for hl in range(2):
                        rows = slice(64 * hl, 64 * hl + 64)
                        nc.tensor.matmul(
                            pst[:, 512 * hl:512 * hl + 512],
                            kt[hp][rows, 128 * i:128 * i + 128],
                            qt[hp][rows, 512 * j:512 * j + 512],
                            start=True, stop=True)
                    ptile = pt_pool.tile([128, 1024], bf16, name="pt")
                    nc.scalar.activation(ptile[:], pst[:], AFT.Exp)
                    if i // 4 == j:
                        o = i % 4
                        nc.vector.tensor_mul(
                            ptile[:], ptile[:],
                            masks[:, 1024 * o:1024 * o + 1024])
                    # interleaved phase-A/C units: land on PE between
                    # the S and AV matmuls, covering the Exp latency.
                    if pending and popped < want_popped(step):
                        pending.pop()()
                        popped += 1
                    if c_pending and step % 5 == 3:
                        c_pending.pop()()
                    for hl in range(2):
                        h = 2 * hp + hl
                        nc.tensor.matmul(
                            po[hl][:], vaug[i][:, 65 * h:65 * h + 65],
                            ptile[:, 512 * hl:512 * hl + 512],
                            start=(i == 0), stop=(i == 4 * j + 3))
                for hl in range(2):
                    rl = rl_pool.tile([1, 512], f32, name="rl")
                    nc.vector.reciprocal(rl[:], po[hl][64:65, :])
                    rlb = rlb_pool.tile([64, 512], f32, name="rlb")
                    nc.gpsimd.partition_broadcast(rlb[:], rl[:])
                    nc.vector.tensor_mul(
                        ot[hp][64 * hl:64 * hl + 64, 512 * j:512 * j + 512],
                        po[hl][0:64, :], rlb[:])
        while pending:
            pending.pop()()
        while c_pending:
            c_pending.pop()()

        if not skew:
            # ---------- phase C inline (repeat==1 path) ----------
            for g in range(8):
                for s in range(NS):
                    ps = pst_pool.tile([128, 1024], f32, name="pst")[:, 0:512]
                    for ci in range(4):
                        nc.tensor.matmul(
                            ps[:], wp_sb[:, C * ci + 128 * g:C * ci + 128 * g + 128],
                            ot[ci][:, 512 * s:512 * s + 512],
                            start=(ci == 0), stop=(ci == 3))
                    yt = yt_pool.tile([128, 512], f32, name="yt")
                    nc.scalar.activation(yt[:], ps[:], AFT.Identity,
                                         bias=bp[:, g:g + 1])
                    nc.gpsimd.dma_start(
                        yt_d.ap()[128 * g:128 * g + 128, 512 * s:512 * s + 512],
                        yt[:])
        yt_pool_cm.__exit__(None, None, None)


def make_inputs(x, W_attn, b_attn, W_proj, b_proj):
    """Host-side sharding: per-core input dicts (bf16 datapath)."""
    import ml_dtypes
    bf = ml_dtypes.bfloat16
    x = np.asarray(x, np.float32)
    W_attn = np.asarray(W_attn, np.float32)
    b_attn = np.asarray(b_attn, np.float32)
    W_proj = np.asarray(W_proj, np.float32)
    b_proj = np.asarray(b_proj, np.float32)

    # masks[kk, 1024*o + 512*hl + qq] = 1 if kk + 128*o <= qq (dup per hl)
    kk = np.arange(128)[:, None]
    qq = np.arange(512)[None, :]
    masks = np.zeros((128, 4 * 1024), np.float32)
    for o in range(4):
        m = (kk + 128 * o <= qq).astype(np.float32)
        masks[:, 1024 * o:1024 * o + 512] = m
        masks[:, 1024 * o + 512:1024 * (o + 1)] = m

    in_maps = []
    for core in range(N_CORES):
        b, g = divmod(core, 2)
        qcols = np.arange(CPC * g, CPC * g + CPC)
        wq = W_attn[:, qcols] * SCALE
        wk = W_attn[:, C + qcols]
        wv = W_attn[:, 2 * C + qcols]
        bq = b_attn[qcols] * SCALE
        bk = b_attn[C + qcols]
        bvv = b_attn[2 * C + qcols]
        bqk = np.concatenate([bq.reshape(4, 128).T, bk.reshape(4, 128).T],
                             axis=1)                     # [128, 8]
        wp = W_proj[CPC * g:CPC * g + CPC, :]
        bpv = (b_proj if g == 0 else np.zeros(C, np.float32))
        in_maps.append({
            "xb": np.ascontiguousarray(x[b].T).astype(bf),
            "wq": np.ascontiguousarray(wq).astype(bf),
            "wk": np.ascontiguousarray(wk).astype(bf),
            "wv": np.ascontiguousarray(wv).astype(bf),
            "bqk": np.ascontiguousarray(bqk),
            "bv": np.ascontiguousarray(bvv.reshape(1, CPC)).astype(bf),
            "wp": np.ascontiguousarray(wp).astype(bf),
            "bp": np.ascontiguousarray(bpv.reshape(8, 128).T),
            "masks": masks.astype(bf),
        })
    return in_maps


def unshard(results):
    """Combine per-core yT partials into [B, T, C] output."""
    out = np.empty((B, T, C), np.float32)
    for b in range(B):
        yt = results[2 * b]["yT"] + results[2 * b + 1]["yT"]
        out[b] = yt.T
    return out


_nc_cache = {}


def kernel(x, W_attn, b_attn, W_proj, b_proj):
    from concourse.bass_utils import run_bass_kernel_spmd
    if "nc" not in _nc_cache:
        _nc_cache["nc"] = build_nc(repeat=1)
    nc = _nc_cache["nc"]
    in_maps = make_inputs(x, W_attn, b_attn, W_proj, b_proj)
    res = run_bass_kernel_spmd(nc, in_maps, core_ids=list(range(N_CORES)),
                               trace=False)
    return unshard(res.results)
